# revision 11
# baseline (speedup 1.0000x reference)
"""Sparse graph attention on 8 Trainium2 NeuronCores (Bass/Tile).

Algorithm (per edge e with dest i0[e], src i1[e]):
    s_e   = dot(q[i0], k[i1]) / 8 + exp(lambda0) * dot(eigs[i0], eigs[i1])
    w_e   = clip(exp(s_e), -5, 5) / denom[i0],  denom[r] = sum_{dest(e)=r} clip(exp(s_e),-5,5)
    out[r] = sum_{dest(e)=r} w_e * v[i1]

Device mapping:
  * Dest-range sharding: core c owns dest rows [c*12500, (c+1)*12500).
  * Edges grouped into "windows": 2048 edge slots from one i1-chunk
    (chunk = 32768 source rows so int16 gather indices fit), <=128 distinct
    dests per window.  Scores are computed all-pairs per window on the PE:
        S^T[slot, r] = B_gT.T @ A_winT     (bf16, fp32 PSUM)
    where B_gT = dma_gather(transpose=True) of packed [k|eigs] rows and
    A_winT holds the window dests' packed [q/8 | eigs*exp(lam)] columns.
  * e = min(exp(S), 5*onehot) -- single tensor_tensor(min) performs the
    mask (oh=0 kills non-dest pairs, even exp=inf), the clip at 5, and
    keeps real scores.  onehot5 built by chained tensor_scalar(is_equal,
    mult) against a materialized iota tile with per-partition dl scalars.
  * Aggregation: psum_out[r, 0:65] += masked_tile.T @ [v|1] rows gathered
    per tile with indirect DMA.  Col 64 accumulates the denominator.
  * psum_out rows scatter-ADDed into a per-core RAW[12544, 65] buffer
    (zero-initialized ExternalOutput); mid-dest window splits are safe.
  * Final pass normalizes RAW -> OUT[12544, 64]; host concatenates slices.
"""

import math
import os
import sys
from contextlib import ExitStack
from dataclasses import dataclass, field

import numpy as np

sys.path.insert(0, "/opt/trn_rl_repo")

import concourse.bass as bass  # noqa: E402
import concourse.mybir as mybir  # noqa: E402
import concourse.tile as tile  # noqa: E402
from concourse import bacc  # noqa: E402
from concourse.library_config import mlp as _mlp_lib  # noqa: E402

F32 = mybir.dt.float32
BF16 = mybir.dt.bfloat16
F16 = mybir.dt.float16
I32 = mybir.dt.int32
I16 = mybir.dt.int16


@dataclass
class Cfg:
    n_nodes: int = 100000
    d_hid: int = 64
    d_eig: int = 32
    n_cores: int = 8
    chunk_rows: int = 32768          # dma_gather int16 index reach
    win_slots: int = 2048            # edge slots per window
    exp_tiles: int = 8               # tiles per exp/mask batch group
    gather_idxs: int = 2048          # idxs per dma_gather call
    # windows per chunk (uniform across cores); set by host prep
    win_sched: tuple = ()            # chunk id per window, e.g. (0,0,...,1,...)
    d_a: int = 96                    # packed score-feature dim (64 q + 32 eigs)
    ke_cols: int = 128               # KE table row length (bf16), 256B rows

    @property
    def tiles_per_win(self):
        return self.win_slots // 128

    @property
    def n_chunks(self):
        return math.ceil(self.n_nodes / self.chunk_rows)

    @property
    def rows_per_core(self):
        return math.ceil(self.n_nodes / self.n_cores)

    @property
    def out_rows(self):             # padded to 128 multiple
        return ((self.rows_per_core + 127) // 128) * 128

    @property
    def n_win(self):
        return len(self.win_sched)


# --------------------------------------------------------------------------
# device program
# --------------------------------------------------------------------------

def emit_kernel(nc: bass.Bass, outs: dict, ins: dict, cfg: Cfg):
    """Emit the full per-core program inside a TileContext."""
    KE = ins["ke"]          # [n_chunks*chunk_rows, 128] f16 (k|eigs|pad)
    VT = ins["vt"]          # [n_chunks*chunk_rows, 128] f16 (v|1|pad)
    AW = ins["aw"]          # [n_win, 96, 128] f16
    KIDX = ins["kidx"]      # [n_win, 128, win_slots//16] i16
    DL = ins["dl"]          # [n_win, 128, tiles_per_win] f32
    OOFF = ins["ooff"]      # [n_win, 128, 1] i32
    IOTA = ins["iota"]      # [128, 128] f32 (row-iota 0..127 per partition)
    RAW = outs["raw"]       # [out_rows, 65] f32 (zero-init)
    OUT = outs["out"]       # [out_rows, 64] f32

    T = cfg.tiles_per_win
    G = cfg.exp_tiles
    n_groups = T // G
    GF = G * 128            # free size of one exp group
    GI = min(cfg.gather_idxs, cfg.win_slots)
    n_gath = cfg.win_slots // GI

    with ExitStack() as ctx:
        tc = ctx.enter_context(tile.TileContext(nc))
        const_p = ctx.enter_context(tc.tile_pool(name="const", bufs=1))
        win_p = ctx.enter_context(tc.tile_pool(name="win", bufs=4))
        big_p = ctx.enter_context(tc.tile_pool(name="big", bufs=3))
        res_p = ctx.enter_context(tc.tile_pool(name="res", bufs=4))
        psS_p = ctx.enter_context(tc.tile_pool(name="psS", bufs=2, space="PSUM"))
        psO_p = ctx.enter_context(tc.tile_pool(name="psO", bufs=2, space="PSUM"))

        nc.gpsimd.load_library(_mlp_lib)

        iota_t = const_p.tile([128, 128], F32)
        nc.sync.dma_start(out=iota_t[:], in_=IOTA[:, :])

        for w, wchunk in enumerate(cfg.win_sched):
            aw_t = win_p.tile([96, 128], F16, tag="aw")
            nc.sync.dma_start(out=aw_t[:], in_=AW[w])
            kidx_t = win_p.tile([128, cfg.win_slots // 16], I16, tag="kidx")
            nc.sync.dma_start(out=kidx_t[:], in_=KIDX[w])
            dl_t = win_p.tile([128, T], F32, tag="dl")
            nc.sync.dma_start(out=dl_t[:], in_=DL[w])
            ooff_t = win_p.tile([128, 1], I32, tag="ooff")
            nc.sync.dma_start(out=ooff_t[:], in_=OOFF[w])

            ke_src = KE[wchunk * cfg.chunk_rows:(wchunk + 1) * cfg.chunk_rows, :]
            vt_src = VT[wchunk * cfg.chunk_rows:(wchunk + 1) * cfg.chunk_rows, :]

            # k|eigs rows gathered transposed: [128 feat, win_slots] f16
            bgt = big_p.tile([128, cfg.win_slots], F16, tag="bgt")
            for gi in range(n_gath):
                nc.gpsimd.dma_gather(
                    bgt[:, gi * GI:(gi + 1) * GI].rearrange(
                        "p (a n) -> p a n", a=1),
                    ke_src,
                    kidx_t[:, gi * (GI // 16):(gi + 1) * (GI // 16)],
                    GI,
                    GI,
                    cfg.ke_cols,
                    transpose=True,
                    single_packet=False,
                )
            # v|1 rows gathered edge-major: [128, T, 128] f16
            vw = big_p.tile([128, T, 128], F16, tag="vw")
            for gi in range(n_gath):
                nc.gpsimd.dma_gather(
                    vw[:, gi * (GI // 128):(gi + 1) * (GI // 128), :],
                    vt_src,
                    kidx_t[:, gi * (GI // 16):(gi + 1) * (GI // 16)],
                    GI,
                    GI,
                    128,
                    single_packet=False,
                )

            psO = psO_p.tile([128, 65], F32, tag="psO")

            for g in range(n_groups):
                psS = psS_p.tile([128, GF], F32, tag="psS")
                for j in range(G):
                    t = g * G + j
                    nc.tensor.matmul(
                        psS[:, j * 128:(j + 1) * 128],
                        lhsT=bgt[0:cfg.d_a, t * 128:(t + 1) * 128],
                        rhs=aw_t[0:cfg.d_a, :],
                        start=True,
                        stop=True,
                    )
                expd = big_p.tile([128, GF], F32, tag="expd")
                nc.scalar.activation(
                    expd[:], psS[:], mybir.ActivationFunctionType.Exp)
                oh5 = big_p.tile([128, GF], F32, tag="oh5")
                for j in range(G):
                    t = g * G + j
                    nc.vector.tensor_scalar(
                        oh5[:, j * 128:(j + 1) * 128],
                        iota_t[:],
                        dl_t[:, t:t + 1],
                        5.0,
                        mybir.AluOpType.is_equal,
                        mybir.AluOpType.mult,
                    )
                # masked weights in f16: min() kills non-dest pairs (incl.
                # exp=inf) and clips at 5 in one op
                masked = big_p.tile([128, GF], F16, tag="masked")
                nc.vector.tensor_tensor(
                    out=masked[:], in0=expd[:], in1=oh5[:],
                    op=mybir.AluOpType.min)
                for j in range(G):
                    t = g * G + j
                    nc.tensor.matmul(
                        psO[:],
                        lhsT=masked[:, j * 128:(j + 1) * 128],
                        rhs=vw[:, t, 0:65],
                        start=(t == 0),
                        stop=(t == T - 1),
                    )

            res = res_p.tile([128, 65], F32, tag="res")
            nc.vector.tensor_copy(out=res[:], in_=psO[:])
            nc.gpsimd.indirect_dma_start(
                out=RAW[:, :],
                out_offset=bass.IndirectOffsetOnAxis(ap=ooff_t[:], axis=0),
                in_=res[:],
                in_offset=None,
                compute_op=mybir.AluOpType.add,
                bounds_check=cfg.out_rows - 1,
                oob_is_err=False,
            )

        # ---- final normalization pass ----
        for i in range(cfg.out_rows // 128):
            raw_t = res_p.tile([128, 65], F32, tag="rawt")
            nc.sync.dma_start(out=raw_t[:], in_=RAW[i * 128:(i + 1) * 128, :])
            den = res_p.tile([128, 1], F32, tag="den")
            nc.vector.tensor_scalar_max(den[:], raw_t[:, 64:65], 1e-30)
            rec = res_p.tile([128, 1], F32, tag="rec")
            nc.vector.reciprocal(rec[:], den[:])
            outt = res_p.tile([128, 64], F32, tag="outt")
            nc.vector.tensor_scalar_mul(outt[:], raw_t[:, 0:64], rec[:])
            nc.sync.dma_start(out=OUT[i * 128:(i + 1) * 128, :], in_=outt[:])


def build_program(cfg: Cfg, debug: bool = False) -> bass.Bass:
    nc = bacc.Bacc("TRN2", target_bir_lowering=False, debug=debug,
                   num_devices=cfg.n_cores)
    n_ke_rows = cfg.n_chunks * cfg.chunk_rows
    ins = {
        "ke": nc.dram_tensor("ke", [n_ke_rows, cfg.ke_cols], F16,
                             kind="ExternalInput").ap(),
        "vt": nc.dram_tensor("vt", [n_ke_rows, 128], F16,
                             kind="ExternalInput").ap(),
        "aw": nc.dram_tensor("aw", [cfg.n_win, 96, 128], F16,
                             kind="ExternalInput").ap(),
        "kidx": nc.dram_tensor("kidx", [cfg.n_win, 128, cfg.win_slots // 16],
                               I16, kind="ExternalInput").ap(),
        "dl": nc.dram_tensor("dl", [cfg.n_win, 128, cfg.tiles_per_win], F32,
                             kind="ExternalInput").ap(),
        "ooff": nc.dram_tensor("ooff", [cfg.n_win, 128, 1], I32,
                               kind="ExternalInput").ap(),
        "iota": nc.dram_tensor("iota", [128, 128], F32,
                               kind="ExternalInput").ap(),
    }
    outs = {
        "raw": nc.dram_tensor("raw", [cfg.out_rows, 65], F32,
                              kind="ExternalOutput").ap(),
        "out": nc.dram_tensor("out", [cfg.out_rows, 64], F32,
                              kind="ExternalOutput").ap(),
    }
    emit_kernel(nc, outs, ins, cfg)
    nc.compile()
    return nc


# --------------------------------------------------------------------------
# host-side data prep
# --------------------------------------------------------------------------

def prep_tables(q, k, v, eigs, lambda0, cfg: Cfg):
    lam = float(np.exp(np.float64(lambda0[0])))
    a_full = np.concatenate(
        [q.astype(np.float32) * np.float32(0.125),
         eigs.astype(np.float32) * np.float32(lam)], axis=1)  # [N, 96]
    a_bf = a_full.astype(np.float16)
    ke = np.zeros((cfg.n_chunks * cfg.chunk_rows, cfg.ke_cols), dtype=a_bf.dtype)
    ke[:cfg.n_nodes, :cfg.d_hid] = k.astype(ke.dtype)
    ke[:cfg.n_nodes, cfg.d_hid:cfg.d_a] = eigs.astype(ke.dtype)
    vt = np.zeros((cfg.n_chunks * cfg.chunk_rows, 128), dtype=np.float16)
    vt[:cfg.n_nodes, :64] = v.astype(np.float16)
    vt[:cfg.n_nodes, 64] = 1.0
    iota = np.broadcast_to(np.arange(128, dtype=np.float32), (128, 128)).copy()
    return a_bf, ke, vt, iota


def _ml_bf16():
    import ml_dtypes
    return ml_dtypes.bfloat16


def build_core_windows(i0_c, i1_c, core_base, cfg: Cfg):
    """Split one core's edges (dest in this core's range) into windows.

    Returns list of windows; each = dict(chunk, dest_list[<=128] (global ids),
    slots: (dl, kidx16, voff) arrays of length win_slots).
    """
    chunk = (i1_c // cfg.chunk_rows).astype(np.int32)
    order = np.lexsort((i0_c, chunk))
    i0_s, i1_s, ch_s = i0_c[order], i1_c[order], chunk[order]
    windows = []
    S = cfg.win_slots
    for c in range(cfg.n_chunks):
        sel = ch_s == c
        d = i0_s[sel]
        s1 = i1_s[sel]
        n = d.shape[0]
        if n == 0:
            continue
        # dest-group boundaries
        newdest = np.empty(n, dtype=bool)
        newdest[0] = True
        np.not_equal(d[1:], d[:-1], out=newdest[1:])
        grp = np.cumsum(newdest) - 1          # dest-group index per edge
        pos = 0
        g0 = 0                                # first dest-group of window
        while pos < n:
            # candidate cut at pos + S
            hi = min(pos + S, n)
            # restrict to <=128 distinct dests
            ghi = grp[hi - 1]
            if ghi - grp[pos] + 1 > 128:
                # find first edge whose group >= grp[pos] + 128
                lim = np.searchsorted(grp, grp[pos] + 128, side="left")
                hi = lim
            cnt = hi - pos
            dests, dl_e = np.unique(d[pos:hi], return_inverse=True)
            windows.append(dict(
                chunk=c,
                dests=dests,
                dl=dl_e.astype(np.float32),
                i1=s1[pos:hi],
            ))
            pos = hi
            g0 = grp[pos] if pos < n else g0
    return windows


def pack_core_inputs(windows, a_bf, core_base, cfg: Cfg):
    """Pack a core's windows (list, possibly shorter than schedule) into
    the static per-chunk schedule given by cfg.win_sched."""
    W = cfg.n_win
    T = cfg.tiles_per_win
    S = cfg.win_slots
    aw = np.zeros((W, 96, 128), dtype=a_bf.dtype)
    kidx = np.zeros((W, 128, S // 16), dtype=np.int16)
    dl = np.full((W, 128, T), -1.0, dtype=np.float32)
    ooff = np.full((W, 128, 1), 1 << 30, dtype=np.int32)

    # slot -> (tile, partition) mapping helpers
    sched = np.asarray(cfg.win_sched)
    by_chunk = {c: np.nonzero(sched == c)[0] for c in range(cfg.n_chunks)}
    used = {c: 0 for c in range(cfg.n_chunks)}
    for win in windows:
        c = win["chunk"]
        wi = by_chunk[c][used[c]]
        used[c] += 1
        nd = win["dests"].shape[0]
        ne = win["i1"].shape[0]
        aw[wi, :, :nd] = a_bf[win["dests"], :].T
        ooff[wi, :nd, 0] = (win["dests"] - core_base).astype(np.int32)
        # per-slot arrays padded to S
        dl_s = np.full(S, -1.0, dtype=np.float32)
        dl_s[:ne] = win["dl"]
        ki_s = np.zeros(S, dtype=np.int16)
        ki_s[:ne] = (win["i1"] - c * cfg.chunk_rows).astype(np.int16)
        dl[wi] = dl_s.reshape(T, 128).T
        # dma_gather wrapped index layout, per 128-idx tile: [128, 8] each
        ki_w = ki_s.reshape(T, 8, 16).transpose(2, 0, 1).reshape(16, T * 8)
        kidx[wi] = np.tile(ki_w, (8, 1))
    return dict(aw=aw, kidx=kidx, dl=dl, ooff=ooff)


def make_schedule(per_core_chunk_counts, cfg: Cfg):
    """Uniform per-chunk window counts = max over cores."""
    need = np.max(np.asarray(per_core_chunk_counts), axis=0)
    sched = []
    for c in range(cfg.n_chunks):
        sched += [c] * int(need[c])
    return tuple(sched)


# --------------------------------------------------------------------------
# top-level entry
# --------------------------------------------------------------------------

_PROGRAM_CACHE: dict = {}
TRACE = False          # set True (e.g. from test.py) to capture an NTFF trace
LAST_EXEC_NS = None    # exec_time_ns of the slowest traced core, if TRACE


def kernel(q, k, v, eigs, lambda0, indices):
    from concourse.bass_utils import run_bass_kernel_spmd

    cfg = Cfg()
    n = cfg.n_nodes
    assert q.shape == (n, cfg.d_hid)

    a_bf, ke, vt, iota = prep_tables(q, k, v, eigs, lambda0, cfg)

    i0 = np.asarray(indices[0], dtype=np.int64)
    i1 = np.asarray(indices[1], dtype=np.int64)
    rows_pc = cfg.rows_per_core
    core_of = (i0 // rows_pc).astype(np.int32)

    all_windows = []
    counts = []
    for c in range(cfg.n_cores):
        m = core_of == c
        wins = build_core_windows(i0[m], i1[m], c * rows_pc, cfg)
        all_windows.append(wins)
        cc = [0] * cfg.n_chunks
        for wdesc in wins:
            cc[wdesc["chunk"]] += 1
        counts.append(cc)

    cfg.win_sched = make_schedule(counts, cfg)

    key = (cfg.win_sched, cfg.out_rows)
    if key not in _PROGRAM_CACHE:
        _PROGRAM_CACHE[key] = build_program(cfg)
    nc = _PROGRAM_CACHE[key]

    in_maps = []
    for c in range(cfg.n_cores):
        packed = pack_core_inputs(all_windows[c], a_bf, c * rows_pc, cfg)
        in_maps.append(dict(
            ke=ke, vt=vt, iota=iota,
            aw=packed["aw"], kidx=packed["kidx"], dl=packed["dl"],
            ooff=packed["ooff"],
        ))

    trace_kw = {}
    if TRACE:
        trace_kw = dict(trace=True, trace_cores=[0])
    res = run_bass_kernel_spmd(nc, in_maps, core_ids=list(range(cfg.n_cores)),
                               **trace_kw)
    global LAST_EXEC_NS
    LAST_EXEC_NS = res.exec_time_ns

    out = np.zeros((n, cfg.d_hid), dtype=np.float32)
    for c in range(cfg.n_cores):
        lo = c * rows_pc
        hi = min(lo + rows_pc, n)
        out[lo:hi] = res.results[c]["out"][:hi - lo]
    return out
